# revision 7
# baseline (speedup 1.0000x reference)
"""Trainium2 Bass kernel for nn_HGNER (windowed bi-LSTM + attention + linear head).

Sharding: 8 cores = (batch row, direction). Each core runs the 4 window LSTMs
of ONE direction over the full 256-token row. This makes sequence-boundary
handling compile-time: invalid window steps are exactly the leading/trailing
token ranges, so state updates are plain sliced writes (no masks, no
copy_predicated). The attention softmax couples the two directions, so the
kernel returns per-direction partial scores s_w = x_half . h_w and partial
logit projections q_w = lin_half @ h_w; the host does the (tiny) softmax and
the final combine in fp32.

Numerics: input projection G = W_ih @ x + b and the recurrent h @ W_hh both
run as fp8(e4m3) DoubleRow matmuls (2 contraction rows/cycle) with f32 PSUM
accumulation; G is stored bf16; gates/activations/elementwise in bf16; h is
stored twice (bf16 for outputs, fp8 for the recurrence). Measured end-to-end
rel err ~7e-3 (threshold 2e-2).

Schedule: windows start staggered (9 at slot 0, 7 at 2, 5 at 4, 3 at 6) so all
end at slot 8; each window runs its two independent 128-token halves
interleaved -> up to 8 concurrent sub-chains keep the Activation engine (the
bottleneck: ~100us of sigmoid/tanh) saturated while PE/DVE/Pool work hides
under it. Gate order is host-permuted to [f,i,o,g] so sigmoid covers chunks
0..8 in one op and tanh covers 9..11.
"""

import numpy as np
import ml_dtypes

import concourse.bass as bass
import concourse.bacc as bacc_mod
import concourse.mybir as mybir
from concourse.tile import TileContext
from concourse.bass_utils import run_bass_kernel_spmd

F32 = mybir.dt.float32
BF16 = mybir.dt.bfloat16
FP8 = mybir.dt.float8e4
AF = mybir.ActivationFunctionType
ALU = mybir.AluOpType
DRMODE = mybir.MatmulPerfMode.DoubleRow

NPF8 = mybir.dt.np(FP8)   # ml_dtypes.float8_e4m3
NPBF = ml_dtypes.bfloat16

B, L, D, H, NW, NL = 4, 256, 768, 384, 4, 9
WINDOWS = (3, 5, 7, 9)
NCORES = 8
HK = 128          # half-token tile
DC = D // 128     # 6 input-feature chunks
HC = H // 128     # 3 hidden chunks
GC = 12           # gate-feature chunks (4H/128)
SCALE = 1.0 / np.sqrt(np.float32(D))

# window start slots chosen so every window ends at slot 9
STARTS = {3: 6, 5: 4, 7: 2, 9: 0}

_CACHE = {}


def _build(dir_id):
    """One direction's program. dir 0 = forward scan, 1 = backward scan.
    SPMD note: all 8 cores run the same image per dir... but SPMD requires ONE
    program for all cores; offsets differ by dir. We therefore build a single
    program parameterized only by DATA, with the direction handled by host-side
    weight/step-order permutation (see _host_prep): a backward window scan over
    offsets [+h..-h] equals a forward scan with G column order reversed and
    the output read reversed. We implement dir=0 order in-kernel; dir=1 cores
    get x (and G inputs) pre-reversed along tokens and their outputs
    un-reversed on the host."""
    nc = bacc_mod.Bacc()

    x8_d = nc.dram_tensor("x8", [D, L], FP8, kind="ExternalInput")
    xh_d = nc.dram_tensor("xh", [H, L], BF16, kind="ExternalInput")
    wih_d = nc.dram_tensor("wih", [NW, D, 4 * H], FP8, kind="ExternalInput")
    whh_d = nc.dram_tensor("whh", [NW, 4 * 128, 4 * H], FP8, kind="ExternalInput")
    bias_d = nc.dram_tensor("bias", [1, NW * 2 * 4 * H], FP8, kind="ExternalInput")
    onesz_d = nc.dram_tensor("onesz", [1, 2 * L], FP8, kind="ExternalInput")
    identb_d = nc.dram_tensor("identb", [128, 128], BF16, kind="ExternalInput")
    onesb_d = nc.dram_tensor("onesb", [128, 1], BF16, kind="ExternalInput")
    linh_d = nc.dram_tensor("linh", [H, NL], BF16, kind="ExternalInput")
    s_d = nc.dram_tensor("s", [128, 8], F32, kind="ExternalOutput")
    q_d = nc.dram_tensor("q", [NL, NW * L], F32, kind="ExternalOutput")

    with TileContext(nc) as tc:
        with (
            tc.tile_pool(name="const", bufs=1) as cpool,
            tc.tile_pool(name="w", bufs=1) as wpool,
            tc.tile_pool(name="g", bufs=1) as g_pool,
            tc.tile_pool(name="st", bufs=1) as st_pool,
            tc.tile_pool(name="act", bufs=6) as act_pool,
            tc.tile_pool(name="tmp", bufs=6) as tmp_pool,
            tc.tile_pool(name="fin", bufs=2) as fin_pool,
            tc.tile_pool(name="ps", bufs=2, space="PSUM") as ps_pool,
        ):
            # ---- constants ----
            x8 = cpool.tile([128, DC * L], FP8, tag="x8")
            nc.sync.dma_start(
                out=x8[:].rearrange("p (k t) -> p k t", k=DC),
                in_=x8_d[:].rearrange("(k p) t -> p k t", p=128),
            )
            xh = cpool.tile([128, HC * L], BF16, tag="xh")
            nc.sync.dma_start(
                out=xh[:].rearrange("p (k t) -> p k t", k=HC),
                in_=xh_d[:].rearrange("(k p) t -> p k t", p=128),
            )
            identb = cpool.tile([128, 128], BF16, tag="identb")
            nc.sync.dma_start(out=identb[:], in_=identb_d[:])
            onesb = cpool.tile([128, 1], BF16, tag="onesb")
            nc.sync.dma_start(out=onesb[:], in_=onesb_d[:])
            onesz = cpool.tile([1, 2 * L], FP8, tag="onesz")
            nc.sync.dma_start(out=onesz[:], in_=onesz_d[:])
            bias8 = cpool.tile([1, NW * 2 * 4 * H], FP8, tag="bias8")
            nc.sync.dma_start(out=bias8[:], in_=bias_d[:])
            linh = cpool.tile([128, HC * NL], BF16, tag="linh")
            nc.sync.dma_start(
                out=linh[:].rearrange("p (k n) -> p k n", k=HC),
                in_=linh_d[:].rearrange("(k p) n -> p k n", p=128),
            )
            # weights, all four windows resident
            wihs, whhs = [], []
            for wi in range(NW):
                wih = wpool.tile([128, DC * 4 * H], FP8, tag=f"wih{wi}")
                nc.sync.dma_start(
                    out=wih[:].rearrange("p (k n) -> p k n", k=DC),
                    in_=wih_d[wi].rearrange("(k p) n -> p k n", p=128),
                )
                wihs.append(wih)
                whh = wpool.tile([128, 4 * 4 * H], FP8, tag=f"whh{wi}")
                nc.sync.dma_start(
                    out=whh[:].rearrange("p (k n) -> p k n", k=4),
                    in_=whh_d[wi].rearrange("(k p) n -> p k n", p=128),
                )
                whhs.append(whh)

            # one-time DVE/Pool touches of DMA'd consts (single-wait collapse)
            wb0 = cpool.tile([128, 1], BF16, tag="wb0")
            nc.vector.tensor_copy(wb0[:], xh[:, 0:1])
            w80 = cpool.tile([128, 1], FP8, tag="w80")
            nc.vector.tensor_copy(w80[:], x8[:, 0:1])

            # ---- per-window state ----
            Gs = {}      # window -> G tile [128, GC*L] bf16
            hsts = {}    # window -> h bf16 [128, HC*L]
            csts = {}    # (window, half) -> c bf16 [128, HC*HK]
            h8s = {}     # (window, half) -> h fp8 [128, 4*HK]

            def emit_g(wi):
                """G = W_ih @ x + b, fp8 DoubleRow, bf16 result."""
                g = g_pool.tile([128, GC * L], BF16, tag=f"g{wi}")
                g3 = g[:].rearrange("p (j t) -> p j t", j=GC)
                wih3 = wihs[wi][:].rearrange("p (k n) -> p k n", k=DC)
                x83 = x8[:].rearrange("p (k t) -> p k t", k=DC)
                bia3 = bias8[:, wi * 2 * 4 * H:(wi + 1) * 2 * 4 * H].rearrange(
                    "o (k n) -> o k n", k=2)
                onz3 = onesz[:].rearrange("o (k t) -> o k t", k=2)
                for jp in range(GC // 2):
                    ps = ps_pool.tile([128, 2 * L], F32, tag="gps")
                    psv = ps[:].rearrange("p (j t) -> p j t", j=2)
                    for ci in range(2):
                        j = jp * 2 + ci
                        for kp in range(DC // 2):
                            nc.tensor.matmul(
                                psv[:, ci, :],
                                lhsT=wih3[:, 2 * kp:2 * kp + 2, j * 128:(j + 1) * 128],
                                rhs=x83[:, 2 * kp:2 * kp + 2, :],
                                start=(kp == 0), stop=False,
                                perf_mode=DRMODE,
                            )
                        nc.tensor.matmul(
                            psv[:, ci, :],
                            lhsT=bia3[:, :, j * 128:(j + 1) * 128],
                            rhs=onz3[:],
                            start=False, stop=True,
                            perf_mode=DRMODE,
                        )
                    nc.vector.tensor_copy(
                        g3[:, jp * 2:jp * 2 + 2, :], psv[:])
                Gs[WINDOWS[wi]] = g

            def emit_substep(w, hb, t):
                wi = WINDOWS.index(w)
                half = w // 2
                o = t - half
                base = hb * HK
                lo = max(0, -o - base)
                hi = min(HK, L - o - base)
                n = hi - lo
                g3 = Gs[w][:].rearrange("p (j t) -> p j t", j=GC)
                gcol = base + lo + o
                hst = hsts[w]
                hv = hst[:].rearrange("p (k t) -> p k t", k=HC)
                cst = csts[(w, hb)]
                cv = cst[:].rearrange("p (k t) -> p k t", k=HC)
                sg = act_pool.tile([128, 9 * HK], BF16, tag="sg")
                sgv = sg[:].rearrange("p (j t) -> p j t", j=9)
                tg = act_pool.tile([128, HC * HK], BF16, tag="tg")
                tgv = tg[:].rearrange("p (j t) -> p j t", j=HC)
                if t == 0:
                    # gates = G directly; f unused (c=0)
                    nc.scalar.activation(
                        sgv[:, 3:9, lo:hi], g3[:, 3:9, gcol:gcol + n], AF.Sigmoid)
                    nc.scalar.activation(
                        tgv[:, :, lo:hi], g3[:, 9:12, gcol:gcol + n], AF.Tanh)
                else:
                    whh3 = whhs[wi][:].rearrange("p (k n) -> p k n", k=4)
                    h83 = h8s[(w, hb)][:].rearrange("p (k t) -> p k t", k=4)
                    gps = ps_pool.tile([128, GC * HK], F32, tag="gates")
                    gpv = gps[:].rearrange("p (j t) -> p j t", j=GC)
                    for j in range(GC):
                        nc.tensor.matmul(
                            gpv[:, j, lo:hi],
                            lhsT=identb[:],
                            rhs=g3[:, j, gcol:gcol + n],
                            start=True, stop=False,
                        )
                        nc.tensor.matmul(
                            gpv[:, j, lo:hi],
                            lhsT=whh3[:, 0:2, j * 128:(j + 1) * 128],
                            rhs=h83[:, 0:2, lo:hi],
                            start=False, stop=False,
                            perf_mode=DRMODE,
                        )
                        nc.tensor.matmul(
                            gpv[:, j, lo:hi],
                            lhsT=whh3[:, 2:4, j * 128:(j + 1) * 128],
                            rhs=h83[:, 2:4, lo:hi],
                            start=False, stop=True,
                            perf_mode=DRMODE,
                        )
                    nc.scalar.activation(
                        sgv[:, 0:9, lo:hi], gpv[:, 0:9, lo:hi], AF.Sigmoid)
                    nc.scalar.activation(
                        tgv[:, :, lo:hi], gpv[:, 9:12, lo:hi], AF.Tanh)
                # c update (gate order [f,i,o,g]: f=0:3, i=3:6, o=6:9)
                if t == 0:
                    nc.vector.tensor_tensor(
                        cv[:, :, lo:hi], sgv[:, 3:6, lo:hi], tgv[:, :, lo:hi],
                        ALU.mult)
                else:
                    fc = tmp_pool.tile([128, HC * HK], BF16, tag="fc")
                    fcv = fc[:].rearrange("p (k t) -> p k t", k=HC)
                    nc.vector.tensor_tensor(
                        fcv[:, :, lo:hi], sgv[:, 0:3, lo:hi], cv[:, :, lo:hi],
                        ALU.mult)
                    ig = tmp_pool.tile([128, HC * HK], BF16, tag="ig")
                    igv = ig[:].rearrange("p (k t) -> p k t", k=HC)
                    nc.vector.tensor_tensor(
                        igv[:, :, lo:hi], sgv[:, 3:6, lo:hi], tgv[:, :, lo:hi],
                        ALU.mult)
                    nc.vector.tensor_tensor(
                        cv[:, :, lo:hi], fcv[:, :, lo:hi], igv[:, :, lo:hi],
                        ALU.add)
                tcn = tmp_pool.tile([128, HC * HK], BF16, tag="tcn")
                tcv = tcn[:].rearrange("p (k t) -> p k t", k=HC)
                nc.scalar.activation(tcv[:, :, lo:hi], cv[:, :, lo:hi], AF.Tanh)
                nc.vector.tensor_tensor(
                    hv[:, :, base + lo:base + hi], sgv[:, 6:9, lo:hi],
                    tcv[:, :, lo:hi], ALU.mult)
                if t < w - 1:
                    h83w = h8s[(w, hb)][:].rearrange("p (k t) -> p k t", k=4)
                    nc.gpsimd.tensor_copy(
                        h83w[:, 0:3, lo:hi], hv[:, :, base + lo:base + hi])

            # ---- staggered schedule ----
            # init state tiles for all (window, half)
            for w in WINDOWS:
                hsts[w] = st_pool.tile([128, HC * L], BF16, tag=f"hst{w}",
                                       name=f"hst{w}")
                for hb in (0, 1):
                    cst = st_pool.tile([128, HC * HK], BF16, tag=f"c{w}_{hb}",
                                       name=f"c{w}_{hb}")
                    nc.vector.memset(cst[:], 0.0)
                    csts[(w, hb)] = cst
                    h8 = st_pool.tile([128, 4 * HK], FP8, tag=f"h8{w}_{hb}",
                                      name=f"h8{w}_{hb}")
                    nc.gpsimd.memset(h8[:], 0.0)
                    h8s[(w, hb)] = h8

            # G emission points: window 9 upfront; 7 after slot 0; 5 after 2;
            # 3 after slot 4
            emit_g(3)  # window index for w=9 is 3
            g_sched = {0: 2, 2: 1, 4: 0}  # slot -> window idx to emit after
            for slot in range(9):
                for w in (9, 7, 5, 3):
                    t = slot - STARTS[w]
                    if 0 <= t < w:
                        for hb in (0, 1):
                            emit_substep(w, hb, t)
                if slot in g_sched:
                    emit_g(g_sched[slot])

            # ---- tail: partial scores s and logit projections q ----
            s_ps = ps_pool.tile([128, 8], F32, tag="gps")
            for wi, w in enumerate(WINDOWS):
                pr = tmp_pool.tile([128, HC * L], BF16, tag="pr")
                nc.vector.tensor_tensor(pr[:], xh[:], hsts[w][:], ALU.mult)
                pv = pr[:].rearrange("p (k t) -> p k t", k=HC)
                for hb in (0, 1):
                    col = wi + 4 * hb
                    for k in range(HC):
                        nc.tensor.matmul(
                            s_ps[:, col:col + 1],
                            lhsT=pv[:, k, hb * HK:(hb + 1) * HK],
                            rhs=onesb[:],
                            start=(k == 0), stop=(k == HC - 1),
                        )
            q_ps = ps_pool.tile([NL, NW * L], F32, tag="gates")
            lv = linh[:].rearrange("p (k n) -> p k n", k=HC)
            for wi, w in enumerate(WINDOWS):
                hv = hsts[w][:].rearrange("p (k t) -> p k t", k=HC)
                for k in range(HC):
                    nc.tensor.matmul(
                        q_ps[:, wi * L:(wi + 1) * L],
                        lhsT=lv[:, k, :],
                        rhs=hv[:, k, :],
                        start=(k == 0), stop=(k == HC - 1),
                    )
            s_sb = fin_pool.tile([128, 8], F32, tag="s_sb")
            nc.vector.tensor_copy(s_sb[:], s_ps[:])
            nc.sync.dma_start(out=s_d[:], in_=s_sb[:])
            q_sb = fin_pool.tile([NL, NW * L], F32, tag="q_sb")
            nc.vector.tensor_copy(q_sb[:], q_ps[:])
            nc.sync.dma_start(out=q_d[:], in_=q_sb[:])

    nc.finalize()
    return nc


def _valid_scatter_np(x, valid_ids):
    Bx, Lx, Dx = x.shape
    v = (valid_ids == 1)
    out = np.zeros_like(x)
    for b in range(Bx):
        sel = x[b][v[b]]
        out[b, :sel.shape[0]] = sel
    return out


# gate permutation torch [i,f,g,o] -> kernel [f,i,o,g]
def _perm_rows():
    idx = np.concatenate([
        np.arange(H, 2 * H),      # f
        np.arange(0, H),          # i
        np.arange(3 * H, 4 * H),  # o
        np.arange(2 * H, 3 * H),  # g
    ])
    return idx


def _host_prep(inputs):
    seq_out = np.asarray(inputs["seq_out"], np.float32)
    valid_ids = np.asarray(inputs["valid_ids"])
    x = _valid_scatter_np(seq_out, valid_ids)  # [B,L,D] f32
    perm = _perm_rows()

    # per-direction weight packs
    packs = {}
    for d, sfx in ((0, "f"), (1, "b")):
        wih = np.empty((NW, D, 4 * H), NPF8)
        whh = np.zeros((NW, 4 * 128, 4 * H), NPF8)
        biasv = np.empty((NW, 2 * 4 * H), NPF8)
        for wi in range(NW):
            wi_p = np.asarray(inputs[f"w_ih_{sfx}"][wi], np.float32)[perm]
            wh_p = np.asarray(inputs[f"w_hh_{sfx}"][wi], np.float32)[perm]
            wih[wi] = wi_p.T.astype(NPF8)
            whh[wi, :H] = wh_p.T.astype(NPF8)
            bv = (np.asarray(inputs[f"b_ih_{sfx}"][wi], np.float32)
                  + np.asarray(inputs[f"b_hh_{sfx}"][wi], np.float32))[perm]
            biasv[wi] = np.tile(bv, 2).astype(NPF8)
        packs[d] = (wih, whh, biasv.reshape(1, -1))

    lin_w = np.asarray(inputs["lin_w"], np.float32)  # [9, 768]
    identb = np.eye(128, dtype=NPBF)
    onesb = np.ones((128, 1), NPBF)
    onesz = np.zeros((1, 2 * L), NPF8)
    onesz[:, :L] = 1.0

    in_maps = []
    for core in range(NCORES):
        b = core // 2
        d = core % 2
        wih, whh, biasv = packs[d]
        xr = x[b]                       # [256, 768]
        if d == 1:
            xr = xr[::-1]               # token-reversed for backward scan
        xh = xr[:, d * H:(d + 1) * H]   # this dir's half for attention
        linh = lin_w[:, d * H:(d + 1) * H]  # [9, 384]
        in_maps.append({
            "x8": np.ascontiguousarray(xr.T).astype(NPF8),
            "xh": np.ascontiguousarray(xh.T).astype(NPBF),
            "wih": wih, "whh": whh, "bias": biasv,
            "onesz": onesz, "identb": identb, "onesb": onesb,
            "linh": np.ascontiguousarray(linh.T).astype(NPBF),
        })
    return x, in_maps


def _combine(x, results):
    """Host-side: softmax over windows + logits assembly, fp32."""
    lin_w = _CACHE["lin_w"]
    lin_b = _CACHE["lin_b"]
    out = np.empty((B, L, NL), np.float32)
    for b in range(B):
        s = np.zeros((NW, L), np.float32)
        q = np.zeros((NW, NL, L), np.float32)
        for d in (0, 1):
            res = results[b * 2 + d]
            sv = np.asarray(res["s"], np.float32)     # [128, 8]
            qv = np.asarray(res["q"], np.float32)     # [9, 4*256]
            qv = qv.reshape(NL, NW, L).transpose(1, 0, 2)  # [NW, NL, L]
            st = np.empty((NW, L), np.float32)
            for wi in range(NW):
                st[wi, :HK] = sv[:, wi]
                st[wi, HK:] = sv[:, wi + 4]
            if d == 1:
                st = st[:, ::-1]
                qv = qv[:, :, ::-1]
            s += st
            q += qv
        sc = s * SCALE
        sc -= sc.max(axis=0, keepdims=True)
        e = np.exp(sc)
        attn = e / e.sum(axis=0, keepdims=True)       # [NW, L]
        local = np.einsum("wl,wnl->ln", attn, q)      # [L, NL]
        r = x[b] @ lin_w.T                            # [L, NL] exact fp32
        out[b] = r + local + lin_b
    return out


def kernel(**inputs) -> np.ndarray:
    if "nc" not in _CACHE:
        _CACHE["nc"] = _build(0)
    nc = _CACHE["nc"]
    _CACHE["lin_w"] = np.asarray(inputs["lin_w"], np.float32)
    _CACHE["lin_b"] = np.asarray(inputs["lin_b"], np.float32)
    x, in_maps = _host_prep(inputs)
    res = run_bass_kernel_spmd(nc, in_maps, core_ids=list(range(NCORES)))
    return _combine(x, res.results)


# revision 24
# speedup vs baseline: 1.2167x; 1.2167x over previous
"""Trainium2 Bass kernel for nn_HGNER (windowed bi-LSTM + attention + linear head).

Sharding: 8 cores = (batch row, direction). Each core runs the 4 window LSTMs
of ONE direction over the full 256-token row. This makes sequence-boundary
handling compile-time: invalid window steps are exactly the leading/trailing
token ranges, so state updates are plain sliced writes (no masks, no
copy_predicated). The attention softmax couples the two directions, so the
kernel returns per-direction partial scores s_w = x_half . h_w and partial
logit projections q_w = lin_half @ h_w; the host does the (tiny) softmax and
the final combine in fp32.

Numerics: input projection G = W_ih @ x + b and the recurrent h @ W_hh both
run as fp8(e4m3) DoubleRow matmuls (2 contraction rows/cycle) with f32 PSUM
accumulation; G is stored bf16; gates/activations/elementwise in bf16; h is
stored twice (bf16 for outputs, fp8 for the recurrence). Measured end-to-end
rel err ~7e-3 (threshold 2e-2).

Schedule: windows start staggered (9 at slot 0, 7 at 2, 5 at 4, 3 at 6) so all
end at slot 8; each window runs its two independent 128-token halves
interleaved -> up to 8 concurrent sub-chains keep the Activation engine (the
bottleneck: ~100us of sigmoid/tanh) saturated while PE/DVE/Pool work hides
under it. Gate order is host-permuted to [f,i,o,g] so sigmoid covers chunks
0..8 in one op and tanh covers 9..11.
"""

import numpy as np
import ml_dtypes

import concourse.bass as bass
import concourse.bacc as bacc_mod
import concourse.mybir as mybir
from concourse.tile import TileContext
from concourse.bass_utils import run_bass_kernel_spmd

F32 = mybir.dt.float32
BF16 = mybir.dt.bfloat16
FP8 = mybir.dt.float8e4
AF = mybir.ActivationFunctionType
ALU = mybir.AluOpType
DRMODE = mybir.MatmulPerfMode.DoubleRow

NPF8 = mybir.dt.np(FP8)   # ml_dtypes.float8_e4m3
NPBF = ml_dtypes.bfloat16

B, L, D, H, NW, NL = 4, 256, 768, 384, 4, 9
WINDOWS = (3, 5, 7, 9)
NCORES = 8
HK = 128          # half-token tile
DC = D // 128     # 6 input-feature chunks
HC = H // 128     # 3 hidden chunks
GC = 12           # gate-feature chunks (4H/128)
SCALE = 1.0 / np.sqrt(np.float32(D))

# window start slots: staggered so G precomputes pipeline in and the two
# mid-size windows end one slot early (their tails hide under slot 8)
STARTS = {3: 6, 5: 3, 7: 1, 9: 0}

_CACHE = {}


def _build(dir_id):
    """One direction's program. dir 0 = forward scan, 1 = backward scan.
    SPMD note: all 8 cores run the same image per dir... but SPMD requires ONE
    program for all cores; offsets differ by dir. We therefore build a single
    program parameterized only by DATA, with the direction handled by host-side
    weight/step-order permutation (see _host_prep): a backward window scan over
    offsets [+h..-h] equals a forward scan with G column order reversed and
    the output read reversed. We implement dir=0 order in-kernel; dir=1 cores
    get x (and G inputs) pre-reversed along tokens and their outputs
    un-reversed on the host."""
    nc = bacc_mod.Bacc()

    x8_d = nc.dram_tensor("x8", [D, L], FP8, kind="ExternalInput")
    xh_d = nc.dram_tensor("xh", [H, L], BF16, kind="ExternalInput")
    wih_d = nc.dram_tensor("wih", [NW, D, 4 * H], FP8, kind="ExternalInput")
    whh_d = nc.dram_tensor("whh", [NW, 4 * 128, 4 * H], FP8, kind="ExternalInput")
    bias_d = nc.dram_tensor("bias", [1, NW * 2 * 4 * H], FP8, kind="ExternalInput")
    onesz_d = nc.dram_tensor("onesz", [1, 2 * L], FP8, kind="ExternalInput")
    identb_d = nc.dram_tensor("identb", [128, 128], BF16, kind="ExternalInput")
    onesb_d = nc.dram_tensor("onesb", [128, 1], BF16, kind="ExternalInput")
    linh_d = nc.dram_tensor("linh", [H, NL], BF16, kind="ExternalInput")
    s_d = nc.dram_tensor("s", [128, 8], F32, kind="ExternalOutput")
    q_d = nc.dram_tensor("q", [NL, NW * L], F32, kind="ExternalOutput")

    with TileContext(nc) as tc:
        with (
            tc.tile_pool(name="const", bufs=1) as cpool,
            tc.tile_pool(name="w", bufs=1) as wpool,
            tc.tile_pool(name="g", bufs=1) as g_pool,
            tc.tile_pool(name="st", bufs=1) as st_pool,
            tc.tile_pool(name="act", bufs=6) as act_pool,
            tc.tile_pool(name="tmp", bufs=6) as tmp_pool,
            tc.tile_pool(name="fin", bufs=2) as fin_pool,
            tc.tile_pool(name="ps", bufs=2, space="PSUM") as ps_pool,
        ):
            # ---- inputs, DMA'd in critical-path order ----
            # wih3 + x8 gate the first G precompute; everything else follows
            # in first-use order (one HWDGE queue, ~0.6us setup per DMA).
            wihs, whhs = [None] * NW, [None] * NW

            def load_wih(wi):
                wih = wpool.tile([128, DC * 4 * H], FP8, tag=f"wih{wi}",
                                 name=f"wih{wi}")
                nc.sync.dma_start(
                    out=wih[:].rearrange("p (k n) -> p k n", k=DC),
                    in_=wih_d[wi].rearrange("(k p) n -> p k n", p=128),
                )
                wihs[wi] = wih

            def load_whh(wi):
                whh = wpool.tile([128, 4 * 4 * H], FP8, tag=f"whh{wi}",
                                 name=f"whh{wi}")
                nc.sync.dma_start(
                    out=whh[:].rearrange("p (k n) -> p k n", k=4),
                    in_=whh_d[wi].rearrange("(k p) n -> p k n", p=128),
                )
                whhs[wi] = whh

            load_wih(3)
            x8 = cpool.tile([128, DC * L], FP8, tag="x8")
            nc.sync.dma_start(
                out=x8[:].rearrange("p (k t) -> p k t", k=DC),
                in_=x8_d[:].rearrange("(k p) t -> p k t", p=128),
            )
            bias8 = cpool.tile([1, NW * 2 * 4 * H], FP8, tag="bias8")
            nc.sync.dma_start(out=bias8[:], in_=bias_d[:])
            onesz = cpool.tile([1, 2 * L], FP8, tag="onesz")
            nc.sync.dma_start(out=onesz[:], in_=onesz_d[:])
            identb = cpool.tile([128, 128], BF16, tag="identb")
            nc.sync.dma_start(out=identb[:], in_=identb_d[:])
            load_whh(3)
            load_wih(2)
            load_whh(2)
            load_wih(1)
            load_whh(1)
            load_wih(0)
            load_whh(0)
            xh = cpool.tile([128, HC * L], BF16, tag="xh")
            nc.sync.dma_start(
                out=xh[:].rearrange("p (k t) -> p k t", k=HC),
                in_=xh_d[:].rearrange("(k p) t -> p k t", p=128),
            )
            onesb = cpool.tile([128, 1], BF16, tag="onesb")
            nc.sync.dma_start(out=onesb[:], in_=onesb_d[:])
            linh = cpool.tile([128, HC * NL], BF16, tag="linh")
            nc.sync.dma_start(
                out=linh[:].rearrange("p (k n) -> p k n", k=HC),
                in_=linh_d[:].rearrange("(k p) n -> p k n", p=128),
            )

            # one-time DVE/Pool touches of DMA'd consts (single-wait collapse)
            wb0 = cpool.tile([128, 1], BF16, tag="wb0")
            nc.vector.tensor_copy(wb0[:], xh[:, 0:1])
            w80 = cpool.tile([128, 1], FP8, tag="w80")
            nc.vector.tensor_copy(w80[:], x8[:, 0:1])

            # PE warmup: the tensor engine runs at reduced clock until it has
            # been continuously busy ~3us; burn that in on scratch matmuls
            # (no DMA dependency) so the first real G matmuls run full-speed.
            scr = cpool.tile([128, 128], BF16, tag="scr")
            nc.vector.memset(scr[:], 0.0)
            warm = ps_pool.tile([128, GC * HK], F32, tag="gates", name="warm")
            wv = warm[:].rearrange("p (j t) -> p j t", j=GC)
            for i in range(36):
                nc.tensor.matmul(
                    wv[:, i % GC, :], lhsT=scr[:], rhs=scr[:],
                    start=True, stop=True,
                )

            # ---- per-window state ----
            Gs = {}      # window -> G tile [128, GC*L] bf16
            hsts = {}    # window -> h bf16 [128, HC*L]
            csts = {}    # (window, half) -> c bf16 [128, HC*HK]
            h8s = {}     # (window, half) -> h fp8 [128, 4*HK]

            def g_gen(wi, c0=0, c1=L):
                """G[:, :, c0:c1] = (W_ih @ x + b)[:, c0:c1], fp8 DoubleRow,
                bf16 result. Generator: yields after each 2-chunk group so
                emission can be interleaved with LSTM sub-steps."""
                w = WINDOWS[wi]
                if w in Gs:
                    g = Gs[w]
                else:
                    g = g_pool.tile([128, GC * L], BF16, tag=f"g{wi}",
                                    name=f"g{wi}")
                    Gs[w] = g
                ncol = c1 - c0
                g3 = g[:].rearrange("p (j t) -> p j t", j=GC)
                wih3 = wihs[wi][:].rearrange("p (k n) -> p k n", k=DC)
                x83 = x8[:].rearrange("p (k t) -> p k t", k=DC)
                bia3 = bias8[:, wi * 2 * 4 * H:(wi + 1) * 2 * 4 * H].rearrange(
                    "o (k n) -> o k n", k=2)
                onz3 = onesz[:].rearrange("o (k t) -> o k t", k=2)
                # chunk-group order: sigma-input chunks (3..8) first so the
                # first sub-step's activation can start before G completes
                for jp in (1, 2, 3, 4, 5, 0):
                    ps = ps_pool.tile([128, 2 * ncol], F32, tag="gps",
                                      name=f"gps{wi}_{jp}")
                    psv = ps[:].rearrange("p (j t) -> p j t", j=2)
                    for ci in range(2):
                        j = jp * 2 + ci
                        for kp in range(DC // 2):
                            nc.tensor.matmul(
                                psv[:, ci, :],
                                lhsT=wih3[:, 2 * kp:2 * kp + 2, j * 128:(j + 1) * 128],
                                rhs=x83[:, 2 * kp:2 * kp + 2, c0:c1],
                                start=(kp == 0), stop=False,
                                perf_mode=DRMODE,
                            )
                        nc.tensor.matmul(
                            psv[:, ci, :],
                            lhsT=bia3[:, :, j * 128:(j + 1) * 128],
                            rhs=onz3[:, :, c0:c1],
                            start=False, stop=True,
                            perf_mode=DRMODE,
                        )
                    nc.vector.tensor_copy(
                        g3[:, jp * 2:jp * 2 + 2, c0:c1], psv[:])
                    yield

            def emit_substep(w, hb, t):
                wi = WINDOWS.index(w)
                half = w // 2
                o = t - half
                base = hb * HK
                lo = max(0, -o - base)
                hi = min(HK, L - o - base)
                n = hi - lo
                g3 = Gs[w][:].rearrange("p (j t) -> p j t", j=GC)
                gcol = base + lo + o
                hst = hsts[w]
                hv = hst[:].rearrange("p (k t) -> p k t", k=HC)
                cst = csts[(w, hb)]
                cv = cst[:].rearrange("p (k t) -> p k t", k=HC)
                sg = act_pool.tile([128, 9 * HK], BF16, tag="sg")
                sgv = sg[:].rearrange("p (j t) -> p j t", j=9)
                tg = act_pool.tile([128, HC * HK], BF16, tag="tg")
                tgv = tg[:].rearrange("p (j t) -> p j t", j=HC)
                if t == 0:
                    # gates = G directly; f unused (c=0)
                    nc.scalar.activation(
                        sgv[:, 3:9, lo:hi], g3[:, 3:9, gcol:gcol + n], AF.Sigmoid)
                    nc.scalar.activation(
                        tgv[:, :, lo:hi], g3[:, 9:12, gcol:gcol + n], AF.Tanh)
                else:
                    whh3 = whhs[wi][:].rearrange("p (k n) -> p k n", k=4)
                    h83 = h8s[(w, hb)][:].rearrange("p (k t) -> p k t", k=4)
                    gps = ps_pool.tile([128, GC * HK], F32, tag="gates")
                    gpv = gps[:].rearrange("p (j t) -> p j t", j=GC)
                    for j in range(GC):
                        nc.tensor.matmul(
                            gpv[:, j, lo:hi],
                            lhsT=identb[:],
                            rhs=g3[:, j, gcol:gcol + n],
                            start=True, stop=False,
                        )
                        nc.tensor.matmul(
                            gpv[:, j, lo:hi],
                            lhsT=whh3[:, 0:2, j * 128:(j + 1) * 128],
                            rhs=h83[:, 0:2, lo:hi],
                            start=False, stop=False,
                            perf_mode=DRMODE,
                        )
                        nc.tensor.matmul(
                            gpv[:, j, lo:hi],
                            lhsT=whh3[:, 2:4, j * 128:(j + 1) * 128],
                            rhs=h83[:, 2:4, lo:hi],
                            start=False, stop=True,
                            perf_mode=DRMODE,
                        )
                    nc.scalar.activation(
                        sgv[:, 0:9, lo:hi], gpv[:, 0:9, lo:hi], AF.Sigmoid)
                    nc.scalar.activation(
                        tgv[:, :, lo:hi], gpv[:, 9:12, lo:hi], AF.Tanh)
                # c update (gate order [f,i,o,g]: f=0:3, i=3:6, o=6:9)
                if t == 0:
                    nc.vector.tensor_tensor(
                        cv[:, :, lo:hi], sgv[:, 3:6, lo:hi], tgv[:, :, lo:hi],
                        ALU.mult)
                else:
                    # f*c on the (otherwise idle) Pool engine
                    fc = tmp_pool.tile([128, HC * HK], BF16, tag="fc")
                    fcv = fc[:].rearrange("p (k t) -> p k t", k=HC)
                    nc.gpsimd.tensor_tensor(
                        fcv[:, :, lo:hi], sgv[:, 0:3, lo:hi], cv[:, :, lo:hi],
                        ALU.mult)
                    ig = tmp_pool.tile([128, HC * HK], BF16, tag="ig")
                    igv = ig[:].rearrange("p (k t) -> p k t", k=HC)
                    nc.vector.tensor_tensor(
                        igv[:, :, lo:hi], sgv[:, 3:6, lo:hi], tgv[:, :, lo:hi],
                        ALU.mult)
                    nc.vector.tensor_tensor(
                        cv[:, :, lo:hi], fcv[:, :, lo:hi], igv[:, :, lo:hi],
                        ALU.add)
                tcn = tmp_pool.tile([128, HC * HK], BF16, tag="tcn")
                tcv = tcn[:].rearrange("p (k t) -> p k t", k=HC)
                nc.scalar.activation(tcv[:, :, lo:hi], cv[:, :, lo:hi], AF.Tanh)
                nc.vector.tensor_tensor(
                    hv[:, :, base + lo:base + hi], sgv[:, 6:9, lo:hi],
                    tcv[:, :, lo:hi], ALU.mult)
                if t < w - 1:
                    h83w = h8s[(w, hb)][:].rearrange("p (k t) -> p k t", k=4)
                    nc.gpsimd.tensor_copy(
                        h83w[:, 0:3, lo:hi], hv[:, :, base + lo:base + hi])

            # ---- staggered schedule ----
            # init state tiles for all (window, half)
            for w in WINDOWS:
                hsts[w] = st_pool.tile([128, HC * L], BF16, tag=f"hst{w}",
                                       name=f"hst{w}")
                for hb in (0, 1):
                    cst = st_pool.tile([128, HC * HK], BF16, tag=f"c{w}_{hb}",
                                       name=f"c{w}_{hb}")
                    nc.vector.memset(cst[:], 0.0)
                    csts[(w, hb)] = cst
                    h8 = st_pool.tile([128, 4 * HK], FP8, tag=f"h8{w}_{hb}",
                                      name=f"h8{w}_{hb}")
                    nc.gpsimd.memset(h8[:], 0.0)
                    h8s[(w, hb)] = h8

            # ---- staggered schedule with interleaved G precompute ----
            lv = linh[:].rearrange("p (k n) -> p k n", k=HC)
            s_sb = fin_pool.tile([128, 8], F32, tag="s_sb")

            def emit_tail_sq(wi, w):
                """Per-window attention partials + logit projection, emitted
                right after the window's last sub-step so they overlap the
                remaining sub-steps' Act work. Uses only transient PSUM tiles
                (accumulates s in SBUF) so nothing holds a rotation buffer."""
                pr = tmp_pool.tile([128, HC * L], BF16, tag="pr")
                nc.vector.tensor_tensor(pr[:], xh[:], hsts[w][:], ALU.mult)
                pv = pr[:].rearrange("p (k t) -> p k t", k=HC)
                s_ps = ps_pool.tile([128, 2], F32, tag="gps",
                                    name=f"s_ps{wi}")
                for hb in (0, 1):
                    for k in range(HC):
                        nc.tensor.matmul(
                            s_ps[:, hb:hb + 1],
                            lhsT=pv[:, k, hb * HK:(hb + 1) * HK],
                            rhs=onesb[:],
                            start=(k == 0), stop=(k == HC - 1),
                        )
                nc.vector.tensor_copy(
                    s_sb[:].rearrange("p (a b) -> p a b", a=2)[:, :, wi],
                    s_ps[:])
                q_ps = ps_pool.tile([NL, L], F32, tag="gps",
                                    name=f"q_ps{wi}")
                hv = hsts[w][:].rearrange("p (k t) -> p k t", k=HC)
                for k in range(HC):
                    nc.tensor.matmul(
                        q_ps[:],
                        lhsT=lv[:, k, :],
                        rhs=hv[:, k, :],
                        start=(k == 0), stop=(k == HC - 1),
                    )
                q_sb = fin_pool.tile([NL, L], F32, tag="q_sb",
                                     name=f"q_sb{wi}", bufs=NW)
                nc.vector.tensor_copy(q_sb[:], q_ps[:])
                nc.sync.dma_start(
                    out=q_d[:, wi * L:(wi + 1) * L], in_=q_sb[:])


            # slot 0: w9's G by column halves so its first sub-steps start
            # early, then w7's G in full (w7 starts at slot 1)
            gens = {}
            for _ in g_gen(3, 0, HK):
                pass
            emit_substep(9, 0, 0)
            for _ in g_gen(3, HK, L):
                pass
            emit_substep(9, 1, 0)
            for _ in g_gen(2):
                pass
            # slot -> (window idx, #groups) of G precompute to interleave
            g_sched = {1: (1, 3), 2: (1, 3), 3: (0, 3), 4: (0, 3)}
            for slot in range(1, 9):
                for w in (9, 7, 5, 3):
                    t = slot - STARTS[w]
                    if 0 <= t < w:
                        for hb in (0, 1):
                            emit_substep(w, hb, t)
                        if t == w - 1:
                            emit_tail_sq(WINDOWS.index(w), w)
                if slot in g_sched:
                    gi, n = g_sched[slot]
                    gen = gens.get(gi)
                    if gen is None:
                        gen = gens[gi] = g_gen(gi)
                    for _ in range(n):
                        next(gen, None)

            nc.sync.dma_start(out=s_d[:], in_=s_sb[:])

    nc.finalize()
    return nc


def _valid_scatter_np(x, valid_ids):
    Bx, Lx, Dx = x.shape
    v = (valid_ids == 1)
    out = np.zeros_like(x)
    for b in range(Bx):
        sel = x[b][v[b]]
        out[b, :sel.shape[0]] = sel
    return out


# gate permutation torch [i,f,g,o] -> kernel [f,i,o,g]
def _perm_rows():
    idx = np.concatenate([
        np.arange(H, 2 * H),      # f
        np.arange(0, H),          # i
        np.arange(3 * H, 4 * H),  # o
        np.arange(2 * H, 3 * H),  # g
    ])
    return idx


def _host_prep(inputs):
    seq_out = np.asarray(inputs["seq_out"], np.float32)
    valid_ids = np.asarray(inputs["valid_ids"])
    x = _valid_scatter_np(seq_out, valid_ids)  # [B,L,D] f32
    perm = _perm_rows()

    # per-direction weight packs
    packs = {}
    for d, sfx in ((0, "f"), (1, "b")):
        wih = np.empty((NW, D, 4 * H), NPF8)
        whh = np.zeros((NW, 4 * 128, 4 * H), NPF8)
        biasv = np.empty((NW, 2 * 4 * H), NPF8)
        for wi in range(NW):
            wi_p = np.asarray(inputs[f"w_ih_{sfx}"][wi], np.float32)[perm]
            wh_p = np.asarray(inputs[f"w_hh_{sfx}"][wi], np.float32)[perm]
            wih[wi] = wi_p.T.astype(NPF8)
            whh[wi, :H] = wh_p.T.astype(NPF8)
            bv = (np.asarray(inputs[f"b_ih_{sfx}"][wi], np.float32)
                  + np.asarray(inputs[f"b_hh_{sfx}"][wi], np.float32))[perm]
            biasv[wi] = np.tile(bv, 2).astype(NPF8)
        packs[d] = (wih, whh, biasv.reshape(1, -1))

    lin_w = np.asarray(inputs["lin_w"], np.float32)  # [9, 768]
    identb = np.eye(128, dtype=NPBF)
    onesb = np.ones((128, 1), NPBF)
    onesz = np.zeros((1, 2 * L), NPF8)
    onesz[:, :L] = 1.0

    in_maps = []
    for core in range(NCORES):
        b = core // 2
        d = core % 2
        wih, whh, biasv = packs[d]
        xr = x[b]                       # [256, 768]
        if d == 1:
            xr = xr[::-1]               # token-reversed for backward scan
        xh = xr[:, d * H:(d + 1) * H]   # this dir's half for attention
        linh = lin_w[:, d * H:(d + 1) * H]  # [9, 384]
        in_maps.append({
            "x8": np.ascontiguousarray(xr.T).astype(NPF8),
            "xh": np.ascontiguousarray(xh.T).astype(NPBF),
            "wih": wih, "whh": whh, "bias": biasv,
            "onesz": onesz, "identb": identb, "onesb": onesb,
            "linh": np.ascontiguousarray(linh.T).astype(NPBF),
        })
    return x, in_maps


def _combine(x, results):
    """Host-side: softmax over windows + logits assembly, fp32."""
    lin_w = _CACHE["lin_w"]
    lin_b = _CACHE["lin_b"]
    out = np.empty((B, L, NL), np.float32)
    for b in range(B):
        s = np.zeros((NW, L), np.float32)
        q = np.zeros((NW, NL, L), np.float32)
        for d in (0, 1):
            res = results[b * 2 + d]
            sv = np.asarray(res["s"], np.float32)     # [128, 8]
            qv = np.asarray(res["q"], np.float32)     # [9, 4*256]
            qv = qv.reshape(NL, NW, L).transpose(1, 0, 2)  # [NW, NL, L]
            st = np.empty((NW, L), np.float32)
            for wi in range(NW):
                st[wi, :HK] = sv[:, wi]
                st[wi, HK:] = sv[:, wi + 4]
            if d == 1:
                st = st[:, ::-1]
                qv = qv[:, :, ::-1]
            s += st
            q += qv
        sc = s * SCALE
        sc -= sc.max(axis=0, keepdims=True)
        e = np.exp(sc)
        attn = e / e.sum(axis=0, keepdims=True)       # [NW, L]
        local = np.einsum("wl,wnl->ln", attn, q)      # [L, NL]
        r = x[b] @ lin_w.T                            # [L, NL] exact fp32
        out[b] = r + local + lin_b
    return out


def kernel(**inputs) -> np.ndarray:
    if "nc" not in _CACHE:
        _CACHE["nc"] = _build(0)
    nc = _CACHE["nc"]
    _CACHE["lin_w"] = np.asarray(inputs["lin_w"], np.float32)
    _CACHE["lin_b"] = np.asarray(inputs["lin_b"], np.float32)
    x, in_maps = _host_prep(inputs)
    res = run_bass_kernel_spmd(nc, in_maps, core_ids=list(range(NCORES)))
    return _combine(x, res.results)
